# revision 7
# baseline (speedup 1.0000x reference)
"""Trainium2 Bass kernel for the low-rank neural operator (nn_LRNO).

Math: reference computes, per layer,
    v1 = einsum('bno,bmoir->bmo', einsum('bnoir,bni->bno', psi, h), phi) / n
with phi/psi produced by per-point DenseNets 2->64->128->256->4096.
Because `temp` carries no (i,r) and `phi` no n, the outer einsum factorizes:
    v1[b,m,o] = T[b,o] * phi_sum[b,m,o] / n
      phi_sum = dense3_phi(a) @ W3p_folded        (fold sum over (i,r) into W3)
      T[b,o]  = sum_{k,i} W3s[k,o,i] M[b,k,i] + sum_i b3s[o,i] hsum[b,i]
      M[b,k,i]= sum_n z_psi[b,n,k] h[b,n,i],  z_psi = dense3_psi(a)
so the 256->4096 layers never materialize.  ~22x fewer FLOPs, verified to
2e-7 rel err against the reference in fp64/fp32.

Sharding: pure data parallel, one batch element per NeuronCore (8 cores).
"""

import sys

try:
    import concourse.bass  # noqa: F401
except ImportError:
    sys.path.insert(0, "/opt/trn_rl_repo")

import numpy as np

NUMI = 8
S = 1024
WIDTH = 32
RANK = 4
BATCH = 8
N_CORES = 8

_BUILT = None


def _layouts():
    # (rows, cols) column-offset maps for the per-layer packs
    r_cols = {
        "W0ps": (0, 2, 0, 128),
        "W1blk": (0, 128, 128, 256),
        "W2p": (0, 128, 384, 256),
        "W3pf": (0, 128, 640, 64),
        "s2W": (0, 128, 704, 256),
        "wWb": (0, 33, 960, 32),
    }
    R_TOT = 992
    f_cols = {
        "b0ps": (0, 128, 0, 1),
        "b1p": (0, 128, 1, 1),
        "b1s": (0, 128, 2, 1),
        "p2b": (0, 128, 3, 2),
        "b3pf": (0, 32, 5, 1),
        "s2b_rep": (0, 128, 6, 256),
        "W3sf": (0, 128, 262, 2048),
        "b3s_io": (0, 32, 2310, 32),
    }
    F_TOT = 2342
    g_cols = {
        "ident_r": (0, 128, 0, 128),
        "fc0Wb": (0, 3, 128, 32),
        "fc1Wb": (0, 33, 160, 64),
        "fc2Wb": (0, 65, 224, 1),
    }
    G_TOT = 225
    return r_cols, R_TOT, f_cols, F_TOT, g_cols, G_TOT


def _build():
    global _BUILT
    if _BUILT is not None:
        return _BUILT

    import concourse.bacc as bacc
    import concourse.mybir as mybir
    from concourse.tile import TileContext

    F32 = mybir.dt.float32
    F32R = mybir.dt.float32r
    ADD = mybir.AluOpType.add
    MULT = mybir.AluOpType.mult
    GELU = mybir.ActivationFunctionType.Gelu
    AXX = mybir.AxisListType.X

    r_cols, R_TOT, f_cols, F_TOT, g_cols, G_TOT = _layouts()

    nc = bacc.Bacc(None, target_bir_lowering=False, debug=False)

    a_d = nc.dram_tensor("aT", (3, S), F32R, kind="ExternalInput")
    gpr_d = nc.dram_tensor("gpr", (128, G_TOT), F32R, kind="ExternalInput")
    gpf_d = nc.dram_tensor("gpf", (128, 128), F32, kind="ExternalInput")
    lpr_d = [nc.dram_tensor(f"lpr{i}", (128, R_TOT), F32R, kind="ExternalInput")
             for i in range(4)]
    lpf_d = [nc.dram_tensor(f"lpf{i}", (128, F_TOT), F32, kind="ExternalInput")
             for i in range(4)]
    out_d = nc.dram_tensor("out", (1, S), F32, kind="ExternalOutput")

    FS = [(0, 512), (512, 512)]  # free-dim splits (offset, size)

    with TileContext(nc) as tc:
        with (
            tc.tile_pool(name="wt", bufs=1) as wt,
            tc.tile_pool(name="work", bufs=4) as wk,
            tc.tile_pool(name="small", bufs=3) as sm,
            tc.tile_pool(name="psA", bufs=2, space="PSUM") as psA,
            tc.tile_pool(name="psB", bufs=2, space="PSUM") as psB,
            tc.tile_pool(name="psC", bufs=1, space="PSUM") as psC,
        ):
            # ---- static tiles + weight DMAs ----
            aT = wt.tile([3, S], F32R, tag="aT")
            gpr = wt.tile([128, G_TOT], F32R, tag="gpr")
            gpf = wt.tile([128, 128], F32, tag="gpf")
            nc.sync.dma_start(aT[:], a_d[:])
            nc.sync.dma_start(gpr[:], gpr_d[:])
            nc.sync.dma_start(gpf[:], gpf_d[:])
            lpr = []
            lpf = []
            for i in range(4):
                tr = wt.tile([128, R_TOT], F32R, tag=f"lpr{i}")
                tf = wt.tile([128, F_TOT], F32, tag=f"lpf{i}")
                nc.sync.dma_start(tr[:], lpr_d[i][:])
                nc.sync.dma_start(tf[:], lpf_d[i][:])
                lpr.append(tr)
                lpf.append(tf)

            def rsl(i, name):
                r0, nr, c0, ncs = r_cols[name]
                return lpr[i][r0:r0 + nr, c0:c0 + ncs]

            def fsl(i, name):
                r0, nr, c0, ncs = f_cols[name]
                return lpf[i][r0:r0 + nr, c0:c0 + ncs]

            def gsl(name):
                r0, nr, c0, ncs = g_cols[name]
                return gpr[r0:r0 + nr, c0:c0 + ncs]

            ident_f = gpf[:, 0:128]

            hA = wt.tile([33, S], F32R, tag="hA")
            hB = wt.tile([33, S], F32R, tag="hB")
            h1 = wt.tile([65, S], F32R, tag="h1")
            nc.vector.memset(hA[32:33, :].bitcast(F32), 1.0)
            nc.vector.memset(hB[32:33, :].bitcast(F32), 1.0)
            nc.vector.memset(h1[64:65, :].bitcast(F32), 1.0)

            def mm_split(out_ps, lhsT, rhs, nparts):
                for o, sz in FS:
                    nc.tensor.matmul(out_ps[0:nparts, o:o + sz], lhsT,
                                     rhs[:, o:o + sz], start=True, stop=True)

            # ---- fc0: h = [fc0W; fc0b]^T @ [a; ones] ----
            h0_ps = psA.tile([128, S], F32, tag="big")
            mm_split(h0_ps, gsl("fc0Wb"), aT, 32)
            nc.vector.tensor_copy(hA[0:32, :], h0_ps[0:32, :])

            hcur = hA
            for li in range(4):
                hnxt = hB if (li % 2 == 0) else hA

                # ======== PART A: h-independent chain work ========
                y0_ps = psA.tile([128, S], F32, tag="big")
                mm_split(y0_ps, rsl(li, "W0ps"), aT[0:2, :], 128)
                y0 = wk.tile([128, S], F32R, tag="act")
                nc.scalar.activation(y0[:], y0_ps[:], GELU, bias=fsl(li, "b0ps"))

                W1 = rsl(li, "W1blk")
                y1s_ps = psA.tile([128, S], F32, tag="big")
                mm_split(y1s_ps, W1[:, 128:256], y0, 128)
                y1s = wk.tile([128, S], F32R, tag="act")
                nc.scalar.activation(y1s[:], y1s_ps[:], GELU, bias=fsl(li, "b1s"))

                y1p_ps = psA.tile([128, S], F32, tag="big")
                mm_split(y1p_ps, W1[:, 0:128], y0, 128)
                y1p = wk.tile([128, S], F32R, tag="act")
                nc.scalar.activation(y1p[:], y1p_ps[:], GELU, bias=fsl(li, "b1p"))

                W2 = rsl(li, "W2p")
                p2b = fsl(li, "p2b")
                y2 = []
                for t in range(2):
                    y2_ps = psA.tile([128, S], F32, tag="big")
                    mm_split(y2_ps, W2[:, 128 * t:128 * (t + 1)], y1p, 128)
                    y2t = wk.tile([128, S], F32R, tag="act")
                    nc.scalar.activation(y2t[:], y2_ps[:], GELU, bias=p2b[:, t:t + 1])
                    y2.append(y2t)

                # z_psi^T blocks: (128n, 256k) each, bias added while staging
                zstage = wk.tile([128, 2048], F32, tag="zst")
                for j in range(8):
                    zps_ps = psB.tile([128, 256], F32, tag="sm")
                    nc.tensor.matmul(zps_ps[:], y1s[:, 128 * j:128 * j + 128],
                                     rsl(li, "s2W"), start=True, stop=True)
                    nc.vector.tensor_tensor(zstage[:, 256 * j:256 * (j + 1)],
                                            zps_ps[:], fsl(li, "s2b_rep"), ADD)
                zpsr = wk.tile([128, 2048], F32R, tag="zpr")
                nc.scalar.activation(zpsr[:], zstage[:], GELU)

                # ======== PART B: h-dependent critical path ========
                hsum = sm.tile([32, 1], F32, tag="hsum")
                nc.vector.reduce_sum(hsum[:], hcur[0:32, :].bitcast(F32), axis=AXX)

                hT_ps = psB.tile([128, 256], F32, tag="sm")
                for j in range(8):
                    nc.tensor.transpose(hT_ps[:, 32 * j:32 * j + 32],
                                        hcur[0:32, 128 * j:128 * j + 128].bitcast(F32),
                                        ident_f[0:32, 0:32])
                hT = sm.tile([128, 256], F32R, tag="hT")
                nc.vector.tensor_copy(hT[:], hT_ps[:])

                # MT[i,k] = sum_n h[i,n] z_psi[k,n]
                MT_ps = psC.tile([32, 1024], F32, tag="acc")
                for j in range(8):
                    nc.tensor.matmul(MT_ps[0:32, 0:256], hT[:, 32 * j:32 * j + 32],
                                     zpsr[:, 256 * j:256 * (j + 1)],
                                     start=(j == 0), stop=(j == 7))
                MTs = sm.tile([32, 256], F32, tag="MTs")
                nc.vector.tensor_copy(MTs[:], MT_ps[0:32, 0:256])

                tp2 = psB.tile([128, 64], F32, tag="sm")
                nc.tensor.transpose(tp2[:, 0:32], MTs[:, 0:128], ident_f[0:32, 0:32])
                nc.tensor.transpose(tp2[:, 32:64], MTs[:, 128:256], ident_f[0:32, 0:32])
                M = sm.tile([128, 64], F32, tag="M")
                nc.vector.tensor_copy(M[:], tp2[:])

                # T[o] = sum_{k,i} W3s[k,o,i] M[k,i] + b3s-term
                ttv = []
                for t in range(2):
                    w3 = fsl(li, "W3sf")[:, 1024 * t:1024 * (t + 1)]
                    prod = wk.tile([128, S], F32, tag="prod")
                    nc.gpsimd.tensor_tensor(
                        prod[:].rearrange("p (o i) -> p o i", i=32),
                        w3.rearrange("p (o i) -> p o i", i=32),
                        M[:, 32 * t:32 * t + 32].unsqueeze(1).broadcast_to((128, 32, 32)),
                        MULT)
                    R = sm.tile([128, 32], F32, tag="R")
                    nc.vector.reduce_sum(R[:],
                                         prod[:].rearrange("p (o i) -> p o i", i=32),
                                         axis=AXX)
                    Rt = psB.tile([32, 128], F32, tag="sm")
                    nc.tensor.transpose(Rt[:], R[:], ident_f)
                    tt = sm.tile([32, 1], F32, tag="tt")
                    nc.vector.reduce_sum(tt[:], Rt[:], axis=AXX)
                    ttv.append(tt)
                b3s_ps = psB.tile([32, 1], F32, tag="sm")
                nc.tensor.matmul(b3s_ps[:], fsl(li, "b3s_io"), hsum[:],
                                 start=True, stop=True)
                tta = sm.tile([32, 1], F32, tag="tta")
                nc.vector.tensor_tensor(tta[:], ttv[0][:], ttv[1][:], ADD)
                Tt = sm.tile([32, 1], F32, tag="Tt")
                nc.vector.tensor_tensor(Tt[:], tta[:], b3s_ps[:], ADD)

                # phi_sum accumulation (reuses the acc slot after MTs copy)
                W3p = rsl(li, "W3pf")
                phi_ps = psC.tile([32, 1024], F32, tag="acc")
                for o, sz in FS:
                    nc.tensor.matmul(phi_ps[0:32, o:o + sz], W3p[:, 0:32],
                                     y2[0][:, o:o + sz], start=True, stop=False)
                    nc.tensor.matmul(phi_ps[0:32, o:o + sz], W3p[:, 32:64],
                                     y2[1][:, o:o + sz], start=False, stop=True)

                # v2 = [w;wb]^T @ [h;1]
                v2_ps = psA.tile([128, S], F32, tag="big")
                mm_split(v2_ps, rsl(li, "wWb"), hcur, 32)
                v2s = sm.tile([32, S], F32, tag="v2s")
                nc.vector.tensor_copy(v2s[:], v2_ps[0:32, :])

                # h_next = (phi_psum + b3pf) * T + v2
                v1 = sm.tile([32, S], F32, tag="v1")
                nc.vector.tensor_scalar(v1[:], phi_ps[0:32, :], fsl(li, "b3pf"),
                                        Tt[:], ADD, MULT)
                nc.vector.tensor_tensor(hnxt[0:32, :], v1[:], v2s[:], ADD)

                hcur = hnxt

            # ---- fc1 (no activation; reference discards the gelu) ----
            f1_ps = psA.tile([128, S], F32, tag="big")
            mm_split(f1_ps, gsl("fc1Wb"), hcur, 64)
            nc.vector.tensor_copy(h1[0:64, :], f1_ps[0:64, :])
            f2_ps = psA.tile([128, S], F32, tag="big")
            mm_split(f2_ps, gsl("fc2Wb"), h1, 1)
            out_sb = wt.tile([1, S], F32, tag="out_sb")
            nc.vector.tensor_copy(out_sb[:], f2_ps[0:1, :])
            nc.sync.dma_start(out_d[:], out_sb[:])

    nc.compile()
    _BUILT = nc
    return nc


def _np(x):
    return np.asarray(x, dtype=np.float32)


def _pack_weights(params):
    r_cols, R_TOT, f_cols, F_TOT, g_cols, G_TOT = _layouts()

    def put(pack, loc, val):
        r0, nr, c0, ncs = loc
        assert val.shape == (nr, ncs), (val.shape, loc)
        pack[r0:r0 + nr, c0:c0 + ncs] = val

    gpr = np.zeros((128, G_TOT), np.float32)
    gpf = np.zeros((128, 128), np.float32)
    eye = np.eye(128, dtype=np.float32)
    put(gpr, g_cols["ident_r"], eye)
    gpf[:, 0:128] = eye
    fc0W, fc0b = _np(params["fc0"][0]), _np(params["fc0"][1])
    put(gpr, g_cols["fc0Wb"], np.vstack([fc0W, fc0b[None, :]]))
    fc1W, fc1b = _np(params["fc1"][0]), _np(params["fc1"][1])
    put(gpr, g_cols["fc1Wb"], np.vstack([fc1W, fc1b[None, :]]))
    fc2W, fc2b = _np(params["fc2"][0]), _np(params["fc2"][1])
    put(gpr, g_cols["fc2Wb"], np.vstack([fc2W, fc2b[None, :]]))

    lprs, lpfs = [], []
    inv_n = 1.0 / S
    for i in range(4):
        phi = [( _np(w), _np(b)) for (w, b) in params["phi"][i]]
        psi = [( _np(w), _np(b)) for (w, b) in params["psi"][i]]
        wW, wb = _np(params["w"][i][0]), _np(params["w"][i][1])

        pr = np.zeros((128, R_TOT), np.float32)
        pf = np.zeros((128, F_TOT), np.float32)

        W0ps = np.zeros((2, 128), np.float32)
        W0ps[:, 0:64] = phi[0][0]
        W0ps[:, 64:128] = psi[0][0]
        put(pr, r_cols["W0ps"], W0ps)
        W1blk = np.zeros((128, 256), np.float32)
        W1blk[0:64, 0:128] = phi[1][0]
        W1blk[64:128, 128:256] = psi[1][0]
        put(pr, r_cols["W1blk"], W1blk)
        put(pr, r_cols["W2p"], phi[2][0])
        # fold phi W3: sum over the 128-wide (i,r) block per o
        W3p = phi[3][0].astype(np.float64)
        W3pf = W3p.reshape(256, WIDTH, WIDTH * RANK).sum(-1).astype(np.float32)
        put(pr, r_cols["W3pf"], np.hstack([W3pf[0:128], W3pf[128:256]]))
        put(pr, r_cols["s2W"], psi[2][0])
        put(pr, r_cols["wWb"], np.vstack([wW, wb[None, :]]))

        b0ps = np.concatenate([phi[0][1], psi[0][1]])
        put(pf, f_cols["b0ps"], b0ps[:, None])
        put(pf, f_cols["b1p"], phi[1][1][:, None])
        put(pf, f_cols["b1s"], psi[1][1][:, None])
        put(pf, f_cols["p2b"], phi[2][1].reshape(2, 128).T)
        b3pf = phi[3][1].astype(np.float64).reshape(WIDTH, WIDTH * RANK).sum(-1)
        put(pf, f_cols["b3pf"], b3pf.astype(np.float32)[:, None])
        put(pf, f_cols["s2b_rep"],
            np.broadcast_to(psi[2][1], (128, 256)).copy())
        W3s = psi[3][0].astype(np.float64)
        W3sf = (W3s.reshape(256, WIDTH * WIDTH, RANK).sum(-1) * inv_n).astype(np.float32)
        put(pf, f_cols["W3sf"], np.hstack([W3sf[0:128], W3sf[128:256]]))
        b3s = psi[3][1].astype(np.float64)
        b3s_oi = (b3s.reshape(WIDTH * WIDTH, RANK).sum(-1) * inv_n).reshape(WIDTH, WIDTH)
        put(pf, f_cols["b3s_io"], b3s_oi.T.astype(np.float32))

        lprs.append(pr)
        lpfs.append(pf)
    return gpr, gpf, lprs, lpfs


def _run(v, params, trace=False, tmpdir=None):
    from concourse.bass_utils import run_bass_kernel_spmd

    nc = _build()
    v = _np(v)
    gpr, gpf, lprs, lpfs = _pack_weights(params)

    grid = np.linspace(0.0, 1.0, S).astype(np.float32)
    in_maps = []
    for c in range(N_CORES):
        aT = np.empty((3, S), np.float32)
        aT[0] = v[c, NUMI:, 0]
        aT[1] = grid
        aT[2] = 1.0
        m = {"aT": aT, "gpr": gpr, "gpf": gpf}
        for i in range(4):
            m[f"lpr{i}"] = lprs[i]
            m[f"lpf{i}"] = lpfs[i]
        in_maps.append(m)

    res = run_bass_kernel_spmd(nc, in_maps, list(range(N_CORES)), trace=trace,
                               tmpdir=tmpdir)
    out = np.stack([res.results[c]["out"].reshape(S, 1) for c in range(N_CORES)])
    return out, res


def kernel(v, params):
    out, _ = _run(v, params, trace=False)
    return out


# revision 11
# speedup vs baseline: 1.0127x; 1.0127x over previous
"""Trainium2 Bass kernel for the low-rank neural operator (nn_LRNO).

Math: reference computes, per layer,
    v1 = einsum('bno,bmoir->bmo', einsum('bnoir,bni->bno', psi, h), phi) / n
with phi/psi produced by per-point DenseNets 2->64->128->256->4096.
Because `temp` carries no (i,r) and `phi` no n, the outer einsum factorizes:
    v1[b,m,o] = T[b,o] * phi_sum[b,m,o] / n
      phi_sum = dense3_phi(a) @ W3p_folded        (fold sum over (i,r) into W3)
      T[b,o]  = sum_{k,i} W3s[k,o,i] M[b,k,i] + sum_i b3s[o,i] hsum[b,i]
      M[b,k,i]= sum_n z_psi[b,n,k] h[b,n,i],  z_psi = dense3_psi(a)
so the 256->4096 layers never materialize.  ~22x fewer FLOPs, verified to
2e-7 rel err against the reference in fp64/fp32.

Sharding: pure data parallel, one batch element per NeuronCore (8 cores).
"""

import sys

try:
    import concourse.bass  # noqa: F401
except ImportError:
    sys.path.insert(0, "/opt/trn_rl_repo")

import numpy as np

NUMI = 8
S = 1024
WIDTH = 32
RANK = 4
BATCH = 8
N_CORES = 8

_BUILT = None


def _layouts():
    # (rows, cols) column-offset maps for the per-layer packs
    r_cols = {
        "W0ps": (0, 2, 0, 128),
        "W1blk": (0, 128, 128, 256),
        "W2p": (0, 128, 384, 256),
        "W3pf": (0, 128, 640, 64),
        "s2W": (0, 128, 704, 256),
        "wWb": (0, 33, 960, 32),
        "b3pf_row": (0, 1, 992, 32),
    }
    R_TOT = 1024
    f_cols = {
        "b0ps": (0, 128, 0, 1),
        "b1p": (0, 128, 1, 1),
        "b1s": (0, 128, 2, 1),
        "p2b": (0, 128, 3, 2),
        "b3pf": (0, 32, 5, 1),
        "s2b_rep": (0, 128, 6, 256),
        "W3sf": (0, 128, 262, 2048),
        "b3s_io": (0, 32, 2310, 32),
    }
    F_TOT = 2342
    g_cols = {
        "ident_r": (0, 128, 0, 128),
        "fc0Wb": (0, 3, 128, 32),
        "fc1Wb": (0, 33, 160, 64),
        "fc2Wb": (0, 65, 224, 1),
        "ones_row": (0, 1, 225, 512),
    }
    G_TOT = 737
    return r_cols, R_TOT, f_cols, F_TOT, g_cols, G_TOT


def _build():
    global _BUILT
    if _BUILT is not None:
        return _BUILT

    import concourse.bacc as bacc
    import concourse.mybir as mybir
    from concourse.tile import TileContext

    F32 = mybir.dt.float32
    F32R = mybir.dt.float32r
    ADD = mybir.AluOpType.add
    MULT = mybir.AluOpType.mult
    GELU = mybir.ActivationFunctionType.Gelu
    AXX = mybir.AxisListType.X

    r_cols, R_TOT, f_cols, F_TOT, g_cols, G_TOT = _layouts()

    nc = bacc.Bacc(None, target_bir_lowering=False, debug=False)

    a_d = nc.dram_tensor("aT", (3, S), F32R, kind="ExternalInput")
    gpr_d = nc.dram_tensor("gpr", (128, G_TOT), F32R, kind="ExternalInput")
    gpf_d = nc.dram_tensor("gpf", (128, 128), F32, kind="ExternalInput")
    lpr_d = [nc.dram_tensor(f"lpr{i}", (128, R_TOT), F32R, kind="ExternalInput")
             for i in range(4)]
    lpf_d = [nc.dram_tensor(f"lpf{i}", (128, F_TOT), F32, kind="ExternalInput")
             for i in range(4)]
    out_d = nc.dram_tensor("out", (1, S), F32, kind="ExternalOutput")

    FS = [(0, 512), (512, 512)]  # free-dim splits (offset, size)

    with TileContext(nc) as tc:
        with (
            tc.tile_pool(name="wt", bufs=1) as wt,
            tc.tile_pool(name="work", bufs=6) as wk,
            tc.tile_pool(name="zw", bufs=2) as zw,
            tc.tile_pool(name="small", bufs=4) as sm,
            tc.tile_pool(name="psA", bufs=2, space="PSUM") as psA,
            tc.tile_pool(name="psSm", bufs=2, space="PSUM") as psSm,
            tc.tile_pool(name="psAcc", bufs=2, space="PSUM") as psAcc,
        ):
            # ---- static tiles + weight DMAs ----
            aT = wt.tile([3, S], F32R, tag="aT")
            gpr = wt.tile([128, G_TOT], F32R, tag="gpr")
            gpf = wt.tile([128, 128], F32, tag="gpf")
            nc.sync.dma_start(aT[:], a_d[:])
            nc.sync.dma_start(gpr[:], gpr_d[:])
            nc.sync.dma_start(gpf[:], gpf_d[:])
            lpr = []
            lpf = []
            for i in range(4):
                tr = wt.tile([128, R_TOT], F32R, tag=f"lpr{i}")
                tf = wt.tile([128, F_TOT], F32, tag=f"lpf{i}")
                nc.sync.dma_start(tr[:], lpr_d[i][:])
                nc.sync.dma_start(tf[:], lpf_d[i][:])
                lpr.append(tr)
                lpf.append(tf)

            def rsl(i, name):
                r0, nr, c0, ncs = r_cols[name]
                return lpr[i][r0:r0 + nr, c0:c0 + ncs]

            def fsl(i, name):
                r0, nr, c0, ncs = f_cols[name]
                return lpf[i][r0:r0 + nr, c0:c0 + ncs]

            def gsl(name):
                r0, nr, c0, ncs = g_cols[name]
                return gpr[r0:r0 + nr, c0:c0 + ncs]

            ident_f = gpf[:, 0:128]

            hA = wt.tile([33, S], F32R, tag="hA")
            hB = wt.tile([33, S], F32R, tag="hB")
            h1 = wt.tile([65, S], F32R, tag="h1")
            nc.vector.memset(hA[32:33, :].bitcast(F32), 1.0)
            nc.vector.memset(hB[32:33, :].bitcast(F32), 1.0)
            nc.vector.memset(h1[64:65, :].bitcast(F32), 1.0)

            def mm_split(out_ps, lhsT, rhs, nparts):
                for o, sz in FS:
                    nc.tensor.matmul(out_ps[0:nparts, o:o + sz], lhsT,
                                     rhs[:, o:o + sz], start=True, stop=True)

            # ---- fc0: h = [fc0W; fc0b]^T @ [a; ones] ----
            h0_ps = psA.tile([128, S], F32, tag="big")
            mm_split(h0_ps, gsl("fc0Wb"), aT, 32)
            nc.vector.tensor_copy(hA[0:32, :], h0_ps[0:32, :])

            hcur = hA
            for li in range(4):
                hnxt = hB if (li % 2 == 0) else hA

                # ======== PART A: h-independent chain work ========
                y0_ps = psA.tile([128, S], F32, tag="big")
                mm_split(y0_ps, rsl(li, "W0ps"), aT[0:2, :], 128)
                y0 = wk.tile([128, S], F32R, tag="act")
                nc.scalar.activation(y0[:], y0_ps[:], GELU, bias=fsl(li, "b0ps"))

                W1 = rsl(li, "W1blk")
                y1s_ps = psA.tile([128, S], F32, tag="big")
                mm_split(y1s_ps, W1[:, 128:256], y0, 128)
                y1s = wk.tile([128, S], F32R, tag="act")
                nc.scalar.activation(y1s[:], y1s_ps[:], GELU, bias=fsl(li, "b1s"))

                # z_psi^T in two 4-block groups; bias added during staging
                zstage = zw.tile([128, 2048], F32, tag="zst")
                for g in range(2):
                    zg_ps = psA.tile([128, S], F32, tag="big")
                    for jj in range(4):
                        j = 4 * g + jj
                        nc.tensor.matmul(zg_ps[:, 256 * jj:256 * (jj + 1)],
                                         y1s[:, 128 * j:128 * j + 128],
                                         rsl(li, "s2W"), start=True, stop=True)
                    nc.vector.tensor_tensor(
                        zstage[:, 1024 * g:1024 * (g + 1)].rearrange(
                            "p (a b) -> p a b", b=256),
                        zg_ps[:].rearrange("p (a b) -> p a b", b=256),
                        fsl(li, "s2b_rep").unsqueeze(1).broadcast_to((128, 4, 256)),
                        ADD)
                zpsr = zw.tile([128, 2048], F32R, tag="zpr", bufs=3)
                nc.scalar.activation(zpsr[:], zstage[:], GELU)

                y1p_ps = psA.tile([128, S], F32, tag="big")
                mm_split(y1p_ps, W1[:, 0:128], y0, 128)
                y1p = wk.tile([128, S], F32R, tag="act")
                nc.scalar.activation(y1p[:], y1p_ps[:], GELU, bias=fsl(li, "b1p"))

                W2 = rsl(li, "W2p")
                p2b = fsl(li, "p2b")
                y2 = []
                for t in range(2):
                    y2_ps = psA.tile([128, S], F32, tag="big")
                    mm_split(y2_ps, W2[:, 128 * t:128 * (t + 1)], y1p, 128)
                    y2t = wk.tile([128, S], F32R, tag="act")
                    nc.scalar.activation(y2t[:], y2_ps[:], GELU, bias=p2b[:, t:t + 1])
                    y2.append(y2t)

                # phi accumulation halves with b3pf folded via K=1 outer product
                W3p = rsl(li, "W3pf")
                phi_h = []
                for o, sz in FS:
                    ph = psAcc.tile([32, 512], F32, tag="acc")
                    nc.tensor.matmul(ph[:], W3p[:, 0:32],
                                     y2[0][:, o:o + sz], start=True, stop=False)
                    nc.tensor.matmul(ph[:], W3p[:, 32:64],
                                     y2[1][:, o:o + sz], start=False, stop=False)
                    nc.tensor.matmul(ph[:], rsl(li, "b3pf_row"),
                                     gsl("ones_row")[:, 0:sz],
                                     start=False, stop=True)
                    phi_h.append(ph)

                # ======== PART B: h-dependent critical path ========
                hsum = sm.tile([32, 1], F32, tag="hsum")
                nc.vector.reduce_sum(hsum[:], hcur[0:32, :].bitcast(F32), axis=AXX)

                hT_ps = psSm.tile([128, 256], F32, tag="sm")
                for j in range(8):
                    nc.tensor.transpose(hT_ps[:, 32 * j:32 * j + 32],
                                        hcur[0:32, 128 * j:128 * j + 128].bitcast(F32),
                                        ident_f[0:32, 0:32])
                hT = sm.tile([128, 256], F32R, tag="hT")
                nc.vector.tensor_copy(hT[:], hT_ps[:])

                # MT[i,k] = sum_n h[i,n] z_psi[k,n]
                MT_ps = psSm.tile([32, 256], F32, tag="sm")
                for j in range(8):
                    nc.tensor.matmul(MT_ps[:], hT[:, 32 * j:32 * j + 32],
                                     zpsr[:, 256 * j:256 * (j + 1)],
                                     start=(j == 0), stop=(j == 7))
                MTs = sm.tile([32, 256], F32, tag="MTs")
                nc.vector.tensor_copy(MTs[:], MT_ps[:])

                tp2 = psSm.tile([128, 64], F32, tag="sm")
                nc.tensor.transpose(tp2[:, 0:32], MTs[:, 0:128], ident_f[0:32, 0:32])
                nc.tensor.transpose(tp2[:, 32:64], MTs[:, 128:256], ident_f[0:32, 0:32])
                M = sm.tile([128, 64], F32, tag="M")
                nc.vector.tensor_copy(M[:], tp2[:])

                # T[o] = sum_{k,i} W3s[k,o,i] M[k,i] + b3s-term
                ttv = []
                for t in range(2):
                    w3 = fsl(li, "W3sf")[:, 1024 * t:1024 * (t + 1)]
                    prod = wk.tile([128, S], F32, tag="prod", bufs=2)
                    eng = nc.gpsimd if t == 0 else nc.vector
                    eng.tensor_tensor(
                        prod[:].rearrange("p (o i) -> p o i", i=32),
                        w3.rearrange("p (o i) -> p o i", i=32),
                        M[:, 32 * t:32 * t + 32].unsqueeze(1).broadcast_to((128, 32, 32)),
                        MULT)
                    R = sm.tile([128, 32], F32, tag="R")
                    nc.vector.reduce_sum(R[:],
                                         prod[:].rearrange("p (o i) -> p o i", i=32),
                                         axis=AXX)
                    Rt = psSm.tile([32, 128], F32, tag="sm")
                    nc.tensor.transpose(Rt[:], R[:], ident_f)
                    tt = sm.tile([32, 1], F32, tag="tt")
                    nc.vector.reduce_sum(tt[:], Rt[:], axis=AXX)
                    ttv.append(tt)
                b3s_ps = psSm.tile([32, 1], F32, tag="sm")
                nc.tensor.matmul(b3s_ps[:], fsl(li, "b3s_io"), hsum[:],
                                 start=True, stop=True)
                tta = sm.tile([32, 1], F32, tag="tta")
                nc.vector.tensor_tensor(tta[:], ttv[0][:], ttv[1][:], ADD)
                Tt = sm.tile([32, 1], F32, tag="Tt")
                nc.vector.tensor_tensor(Tt[:], tta[:], b3s_ps[:], ADD)

                # h_next = phi_b * T + v2, via in-place scale then PE accumulate
                for idx, (o, sz) in enumerate(FS):
                    ph = phi_h[idx]
                    nc.vector.tensor_scalar(ph[:], ph[:], Tt[:], None, MULT)
                    nc.tensor.matmul(ph[:], rsl(li, "wWb"), hcur[:, o:o + sz],
                                     start=False, stop=True)
                    nc.vector.tensor_copy(hnxt[0:32, o:o + sz], ph[:])

                hcur = hnxt

            # ---- fc1 (no activation; reference discards the gelu) ----
            f1_ps = psA.tile([128, S], F32, tag="big")
            mm_split(f1_ps, gsl("fc1Wb"), hcur, 64)
            nc.vector.tensor_copy(h1[0:64, :], f1_ps[0:64, :])
            f2_ps = psA.tile([128, S], F32, tag="big")
            mm_split(f2_ps, gsl("fc2Wb"), h1, 1)
            out_sb = wt.tile([1, S], F32, tag="out_sb")
            nc.vector.tensor_copy(out_sb[:], f2_ps[0:1, :])
            nc.sync.dma_start(out_d[:], out_sb[:])

    nc.compile()
    _BUILT = nc
    return nc


def _np(x):
    return np.asarray(x, dtype=np.float32)


def _pack_weights(params):
    r_cols, R_TOT, f_cols, F_TOT, g_cols, G_TOT = _layouts()

    def put(pack, loc, val):
        r0, nr, c0, ncs = loc
        assert val.shape == (nr, ncs), (val.shape, loc)
        pack[r0:r0 + nr, c0:c0 + ncs] = val

    gpr = np.zeros((128, G_TOT), np.float32)
    gpf = np.zeros((128, 128), np.float32)
    eye = np.eye(128, dtype=np.float32)
    put(gpr, g_cols["ident_r"], eye)
    gpf[:, 0:128] = eye
    fc0W, fc0b = _np(params["fc0"][0]), _np(params["fc0"][1])
    put(gpr, g_cols["fc0Wb"], np.vstack([fc0W, fc0b[None, :]]))
    fc1W, fc1b = _np(params["fc1"][0]), _np(params["fc1"][1])
    put(gpr, g_cols["fc1Wb"], np.vstack([fc1W, fc1b[None, :]]))
    fc2W, fc2b = _np(params["fc2"][0]), _np(params["fc2"][1])
    put(gpr, g_cols["fc2Wb"], np.vstack([fc2W, fc2b[None, :]]))
    put(gpr, g_cols["ones_row"], np.ones((1, 512), np.float32))

    lprs, lpfs = [], []
    inv_n = 1.0 / S
    for i in range(4):
        phi = [( _np(w), _np(b)) for (w, b) in params["phi"][i]]
        psi = [( _np(w), _np(b)) for (w, b) in params["psi"][i]]
        wW, wb = _np(params["w"][i][0]), _np(params["w"][i][1])

        pr = np.zeros((128, R_TOT), np.float32)
        pf = np.zeros((128, F_TOT), np.float32)

        W0ps = np.zeros((2, 128), np.float32)
        W0ps[:, 0:64] = phi[0][0]
        W0ps[:, 64:128] = psi[0][0]
        put(pr, r_cols["W0ps"], W0ps)
        W1blk = np.zeros((128, 256), np.float32)
        W1blk[0:64, 0:128] = phi[1][0]
        W1blk[64:128, 128:256] = psi[1][0]
        put(pr, r_cols["W1blk"], W1blk)
        put(pr, r_cols["W2p"], phi[2][0])
        # fold phi W3: sum over the 128-wide (i,r) block per o
        W3p = phi[3][0].astype(np.float64)
        W3pf = W3p.reshape(256, WIDTH, WIDTH * RANK).sum(-1).astype(np.float32)
        put(pr, r_cols["W3pf"], np.hstack([W3pf[0:128], W3pf[128:256]]))
        put(pr, r_cols["s2W"], psi[2][0])
        put(pr, r_cols["wWb"], np.vstack([wW, wb[None, :]]))

        b0ps = np.concatenate([phi[0][1], psi[0][1]])
        put(pf, f_cols["b0ps"], b0ps[:, None])
        put(pf, f_cols["b1p"], phi[1][1][:, None])
        put(pf, f_cols["b1s"], psi[1][1][:, None])
        put(pf, f_cols["p2b"], phi[2][1].reshape(2, 128).T)
        b3pf = phi[3][1].astype(np.float64).reshape(WIDTH, WIDTH * RANK).sum(-1)
        put(pf, f_cols["b3pf"], b3pf.astype(np.float32)[:, None])
        put(pr, r_cols["b3pf_row"], b3pf.astype(np.float32)[None, :])
        put(pf, f_cols["s2b_rep"],
            np.broadcast_to(psi[2][1], (128, 256)).copy())
        W3s = psi[3][0].astype(np.float64)
        W3sf = (W3s.reshape(256, WIDTH * WIDTH, RANK).sum(-1) * inv_n).astype(np.float32)
        put(pf, f_cols["W3sf"], np.hstack([W3sf[0:128], W3sf[128:256]]))
        b3s = psi[3][1].astype(np.float64)
        b3s_oi = (b3s.reshape(WIDTH * WIDTH, RANK).sum(-1) * inv_n).reshape(WIDTH, WIDTH)
        put(pf, f_cols["b3s_io"], b3s_oi.T.astype(np.float32))

        lprs.append(pr)
        lpfs.append(pf)
    return gpr, gpf, lprs, lpfs


def _run(v, params, trace=False, tmpdir=None):
    from concourse.bass_utils import run_bass_kernel_spmd

    nc = _build()
    v = _np(v)
    gpr, gpf, lprs, lpfs = _pack_weights(params)

    grid = np.linspace(0.0, 1.0, S).astype(np.float32)
    in_maps = []
    for c in range(N_CORES):
        aT = np.empty((3, S), np.float32)
        aT[0] = v[c, NUMI:, 0]
        aT[1] = grid
        aT[2] = 1.0
        m = {"aT": aT, "gpr": gpr, "gpf": gpf}
        for i in range(4):
            m[f"lpr{i}"] = lprs[i]
            m[f"lpf{i}"] = lpfs[i]
        in_maps.append(m)

    res = run_bass_kernel_spmd(nc, in_maps, list(range(N_CORES)), trace=trace,
                               tmpdir=tmpdir)
    out = np.stack([res.results[c]["out"].reshape(S, 1) for c in range(N_CORES)])
    return out, res


def kernel(v, params):
    out, _ = _run(v, params, trace=False)
    return out


# revision 12
# speedup vs baseline: 1.0727x; 1.0593x over previous
"""Trainium2 Bass kernel for the low-rank neural operator (nn_LRNO).

Math: reference computes, per layer,
    v1 = einsum('bno,bmoir->bmo', einsum('bnoir,bni->bno', psi, h), phi) / n
with phi/psi produced by per-point DenseNets 2->64->128->256->4096.
Because `temp` carries no (i,r) and `phi` no n, the outer einsum factorizes:
    v1[b,m,o] = T[b,o] * phi_sum[b,m,o] / n
      phi_sum = dense3_phi(a) @ W3p_folded        (fold sum over (i,r) into W3)
      T[b,o]  = sum_{k,i} W3s[k,o,i] M[b,k,i] + sum_i b3s[o,i] hsum[b,i]
      M[b,k,i]= sum_n z_psi[b,n,k] h[b,n,i],  z_psi = dense3_psi(a)
so the 256->4096 layers never materialize.  ~22x fewer FLOPs, verified to
2e-7 rel err against the reference in fp64/fp32.

Sharding: pure data parallel, one batch element per NeuronCore (8 cores).
"""

import sys

try:
    import concourse.bass  # noqa: F401
except ImportError:
    sys.path.insert(0, "/opt/trn_rl_repo")

import numpy as np

NUMI = 8
S = 1024
WIDTH = 32
RANK = 4
BATCH = 8
N_CORES = 8

_BUILT = None


def _layouts():
    # (rows, cols) column-offset maps for the per-layer packs
    r_cols = {
        "W0ps": (0, 2, 0, 128),
        "W1blk": (0, 128, 128, 256),
        "W2p": (0, 128, 384, 256),
        "W3pf": (0, 128, 640, 64),
        "s2W": (0, 128, 704, 256),
        "wWb": (0, 33, 960, 32),
        "b3pf_row": (0, 1, 992, 32),
    }
    R_TOT = 1024
    f_cols = {
        "b0ps": (0, 128, 0, 1),
        "b1p": (0, 128, 1, 1),
        "b1s": (0, 128, 2, 1),
        "p2b": (0, 128, 3, 2),
        "b3pf": (0, 32, 5, 1),
        "s2b_rep": (0, 128, 6, 256),
        "W3sf": (0, 128, 262, 2048),
        "b3s_io": (0, 32, 2310, 32),
    }
    F_TOT = 2342
    g_cols = {
        "ident_r": (0, 128, 0, 128),
        "fc0Wb": (0, 3, 128, 32),
        "fc1Wb": (0, 33, 160, 64),
        "fc2Wb": (0, 65, 224, 1),
        "ones_row": (0, 1, 225, 512),
    }
    G_TOT = 737
    return r_cols, R_TOT, f_cols, F_TOT, g_cols, G_TOT


def _build():
    global _BUILT
    if _BUILT is not None:
        return _BUILT

    import concourse.bacc as bacc
    import concourse.mybir as mybir
    from concourse.tile import TileContext

    F32 = mybir.dt.float32
    F32R = mybir.dt.float32r
    ADD = mybir.AluOpType.add
    MULT = mybir.AluOpType.mult
    GELU = mybir.ActivationFunctionType.Gelu
    AXX = mybir.AxisListType.X

    r_cols, R_TOT, f_cols, F_TOT, g_cols, G_TOT = _layouts()

    nc = bacc.Bacc(None, target_bir_lowering=False, debug=False)

    a_d = nc.dram_tensor("aT", (3, S), F32R, kind="ExternalInput")
    gpr_d = nc.dram_tensor("gpr", (128, G_TOT), F32R, kind="ExternalInput")
    gpf_d = nc.dram_tensor("gpf", (128, 128), F32, kind="ExternalInput")
    lpr_d = [nc.dram_tensor(f"lpr{i}", (128, R_TOT), F32R, kind="ExternalInput")
             for i in range(4)]
    lpf_d = [nc.dram_tensor(f"lpf{i}", (128, F_TOT), F32, kind="ExternalInput")
             for i in range(4)]
    out_d = nc.dram_tensor("out", (1, S), F32, kind="ExternalOutput")

    FS = [(0, 512), (512, 512)]  # free-dim splits (offset, size)

    with TileContext(nc) as tc:
        with (
            tc.tile_pool(name="wt", bufs=1) as wt,
            tc.tile_pool(name="work", bufs=6) as wk,
            tc.tile_pool(name="zw", bufs=2) as zw,
            tc.tile_pool(name="small", bufs=4) as sm,
            tc.tile_pool(name="psA", bufs=2, space="PSUM") as psA,
            tc.tile_pool(name="psSm", bufs=2, space="PSUM") as psSm,
            tc.tile_pool(name="psAcc", bufs=2, space="PSUM") as psAcc,
        ):
            # ---- static tiles + weight DMAs ----
            aT = wt.tile([3, S], F32R, tag="aT")
            gpr = wt.tile([128, G_TOT], F32R, tag="gpr")
            gpf = wt.tile([128, 128], F32, tag="gpf")
            nc.sync.dma_start(aT[:], a_d[:])
            nc.sync.dma_start(gpr[:], gpr_d[:])
            nc.sync.dma_start(gpf[:], gpf_d[:])
            lpr = []
            lpf = []
            for i in range(4):
                tr = wt.tile([128, R_TOT], F32R, tag=f"lpr{i}")
                tf = wt.tile([128, F_TOT], F32, tag=f"lpf{i}")
                nc.sync.dma_start(tr[:], lpr_d[i][:])
                nc.sync.dma_start(tf[:], lpf_d[i][:])
                lpr.append(tr)
                lpf.append(tf)

            def rsl(i, name):
                r0, nr, c0, ncs = r_cols[name]
                return lpr[i][r0:r0 + nr, c0:c0 + ncs]

            def fsl(i, name):
                r0, nr, c0, ncs = f_cols[name]
                return lpf[i][r0:r0 + nr, c0:c0 + ncs]

            def gsl(name):
                r0, nr, c0, ncs = g_cols[name]
                return gpr[r0:r0 + nr, c0:c0 + ncs]

            ident_f = gpf[:, 0:128]
            ident_r = gsl("ident_r")

            hA = wt.tile([33, S], F32R, tag="hA")
            hB = wt.tile([33, S], F32R, tag="hB")
            h1 = wt.tile([65, S], F32R, tag="h1")
            nc.vector.memset(hA[32:33, :].bitcast(F32), 1.0)
            nc.vector.memset(hB[32:33, :].bitcast(F32), 1.0)
            nc.vector.memset(h1[64:65, :].bitcast(F32), 1.0)

            def mm_split(out_ps, lhsT, rhs, nparts):
                for o, sz in FS:
                    nc.tensor.matmul(out_ps[0:nparts, o:o + sz], lhsT,
                                     rhs[:, o:o + sz], start=True, stop=True)

            # ---- fc0: h = [fc0W; fc0b]^T @ [a; ones] ----
            h0_ps = psA.tile([128, S], F32, tag="big")
            mm_split(h0_ps, gsl("fc0Wb"), aT, 32)
            nc.vector.tensor_copy(hA[0:32, :], h0_ps[0:32, :])

            hcur = hA
            for li in range(4):
                hnxt = hB if (li % 2 == 0) else hA

                # ======== PART A: h-independent chain work ========
                y0_ps = psA.tile([128, S], F32, tag="big")
                mm_split(y0_ps, rsl(li, "W0ps"), aT[0:2, :], 128)
                y0 = wk.tile([128, S], F32R, tag="act")
                nc.scalar.activation(y0[:], y0_ps[:], GELU, bias=fsl(li, "b0ps"))

                W1 = rsl(li, "W1blk")
                y1s_ps = psA.tile([128, S], F32, tag="big")
                mm_split(y1s_ps, W1[:, 128:256], y0, 128)
                y1s = wk.tile([128, S], F32R, tag="act")
                nc.scalar.activation(y1s[:], y1s_ps[:], GELU, bias=fsl(li, "b1s"))

                # z_psi^T in two 4-block groups; bias added during staging
                zstage = zw.tile([128, 2048], F32, tag="zst")
                for g in range(2):
                    zg_ps = psA.tile([128, S], F32, tag="big")
                    for jj in range(4):
                        j = 4 * g + jj
                        nc.tensor.matmul(zg_ps[:, 256 * jj:256 * (jj + 1)],
                                         y1s[:, 128 * j:128 * j + 128],
                                         rsl(li, "s2W"), start=True, stop=True)
                    nc.vector.tensor_tensor(
                        zstage[:, 1024 * g:1024 * (g + 1)].rearrange(
                            "p (a b) -> p a b", b=256),
                        zg_ps[:].rearrange("p (a b) -> p a b", b=256),
                        fsl(li, "s2b_rep").unsqueeze(1).broadcast_to((128, 4, 256)),
                        ADD)
                zpsr = zw.tile([128, 2048], F32R, tag="zpr", bufs=3)
                nc.scalar.activation(zpsr[:], zstage[:], GELU)

                y1p_ps = psA.tile([128, S], F32, tag="big")
                mm_split(y1p_ps, W1[:, 0:128], y0, 128)
                y1p = wk.tile([128, S], F32R, tag="act")
                nc.scalar.activation(y1p[:], y1p_ps[:], GELU, bias=fsl(li, "b1p"))

                W2 = rsl(li, "W2p")
                p2b = fsl(li, "p2b")
                y2 = []
                for t in range(2):
                    y2_ps = psA.tile([128, S], F32, tag="big")
                    mm_split(y2_ps, W2[:, 128 * t:128 * (t + 1)], y1p, 128)
                    y2t = wk.tile([128, S], F32R, tag="act")
                    nc.scalar.activation(y2t[:], y2_ps[:], GELU, bias=p2b[:, t:t + 1])
                    y2.append(y2t)

                # phi accumulation halves with b3pf folded via K=1 outer product
                W3p = rsl(li, "W3pf")
                phi_h = []
                for o, sz in FS:
                    ph = psAcc.tile([32, 512], F32, tag="acc")
                    nc.tensor.matmul(ph[:], W3p[:, 0:32],
                                     y2[0][:, o:o + sz], start=True, stop=False)
                    nc.tensor.matmul(ph[:], W3p[:, 32:64],
                                     y2[1][:, o:o + sz], start=False, stop=False)
                    nc.tensor.matmul(ph[:], rsl(li, "b3pf_row"),
                                     gsl("ones_row")[:, 0:sz],
                                     start=False, stop=True)
                    phi_h.append(ph)

                # ======== PART B: h-dependent critical path ========
                hsum = sm.tile([32, 1], F32, tag="hsum")
                nc.vector.reduce_sum(hsum[:], hcur[0:32, :].bitcast(F32), axis=AXX)

                v2s = sm.tile([32, S], F32, tag="v2s")
                for o, sz in FS:
                    v2_ps = psSm.tile([32, 512], F32, tag="sm")
                    nc.tensor.matmul(v2_ps[:], rsl(li, "wWb"), hcur[:, o:o + sz],
                                     start=True, stop=True)
                    nc.vector.tensor_copy(v2s[:, o:o + sz], v2_ps[:])

                hT_ps = psSm.tile([128, 256], F32R, tag="sm")
                for j in range(8):
                    nc.tensor.transpose(hT_ps[:, 32 * j:32 * j + 32],
                                        hcur[0:32, 128 * j:128 * j + 128],
                                        ident_r[0:32, 0:32])
                hT = sm.tile([128, 256], F32R, tag="hT")
                nc.vector.tensor_copy(hT[:], hT_ps[:])

                # MT[i,k] = sum_n h[i,n] z_psi[k,n]
                MT_ps = psSm.tile([32, 256], F32, tag="sm")
                for j in range(8):
                    nc.tensor.matmul(MT_ps[:], hT[:, 32 * j:32 * j + 32],
                                     zpsr[:, 256 * j:256 * (j + 1)],
                                     start=(j == 0), stop=(j == 7))
                MTs = sm.tile([32, 256], F32, tag="MTs")
                nc.vector.tensor_copy(MTs[:], MT_ps[:])

                tp2 = psSm.tile([128, 64], F32, tag="sm")
                nc.tensor.transpose(tp2[:, 0:32], MTs[:, 0:128], ident_f[0:32, 0:32])
                nc.tensor.transpose(tp2[:, 32:64], MTs[:, 128:256], ident_f[0:32, 0:32])
                M = sm.tile([128, 64], F32, tag="M")
                nc.vector.tensor_copy(M[:], tp2[:])

                # T[o] = sum_{k,i} W3s[k,o,i] M[k,i] + b3s-term
                w3 = fsl(li, "W3sf")
                prod = wk.tile([128, 2048], F32, tag="prod", bufs=2)
                nc.vector.tensor_tensor(
                    prod[:].rearrange("p (t o i) -> p t o i", t=2, i=32),
                    w3.rearrange("p (t o i) -> p t o i", t=2, i=32),
                    M[:].rearrange("p (t i) -> p t i", t=2).unsqueeze(2)
                        .broadcast_to((128, 2, 32, 32)),
                    MULT)
                R = sm.tile([128, 64], F32, tag="R")
                nc.vector.reduce_sum(R[:],
                                     prod[:].rearrange("p (a i) -> p a i", i=32),
                                     axis=AXX)
                Rt = psSm.tile([32, 256], F32, tag="sm")
                nc.tensor.transpose(Rt[:, 0:128], R[:, 0:32], ident_f)
                nc.tensor.transpose(Rt[:, 128:256], R[:, 32:64], ident_f)
                b3s_ps = psSm.tile([32, 1], F32, tag="sm")
                nc.tensor.matmul(b3s_ps[:], fsl(li, "b3s_io"), hsum[:],
                                 start=True, stop=True)
                tta = sm.tile([32, 1], F32, tag="tta")
                nc.vector.reduce_sum(tta[:], Rt[:], axis=AXX)
                Tt = sm.tile([32, 1], F32, tag="Tt")
                nc.vector.tensor_tensor(Tt[:], tta[:], b3s_ps[:], ADD)

                # h_next = phi_b * T + v2 (single fused DVE op per half)
                for idx, (o, sz) in enumerate(FS):
                    nc.vector.scalar_tensor_tensor(
                        hnxt[0:32, o:o + sz], phi_h[idx][:], Tt[:],
                        v2s[:, o:o + sz], MULT, ADD)

                hcur = hnxt

            # ---- fc1 (no activation; reference discards the gelu) ----
            f1_ps = psA.tile([128, S], F32, tag="big")
            mm_split(f1_ps, gsl("fc1Wb"), hcur, 64)
            nc.vector.tensor_copy(h1[0:64, :], f1_ps[0:64, :])
            f2_ps = psA.tile([128, S], F32, tag="big")
            mm_split(f2_ps, gsl("fc2Wb"), h1, 1)
            out_sb = wt.tile([1, S], F32, tag="out_sb")
            nc.vector.tensor_copy(out_sb[:], f2_ps[0:1, :])
            nc.sync.dma_start(out_d[:], out_sb[:])

    nc.compile()
    _BUILT = nc
    return nc


def _np(x):
    return np.asarray(x, dtype=np.float32)


def _pack_weights(params):
    r_cols, R_TOT, f_cols, F_TOT, g_cols, G_TOT = _layouts()

    def put(pack, loc, val):
        r0, nr, c0, ncs = loc
        assert val.shape == (nr, ncs), (val.shape, loc)
        pack[r0:r0 + nr, c0:c0 + ncs] = val

    gpr = np.zeros((128, G_TOT), np.float32)
    gpf = np.zeros((128, 128), np.float32)
    eye = np.eye(128, dtype=np.float32)
    put(gpr, g_cols["ident_r"], eye)
    gpf[:, 0:128] = eye
    fc0W, fc0b = _np(params["fc0"][0]), _np(params["fc0"][1])
    put(gpr, g_cols["fc0Wb"], np.vstack([fc0W, fc0b[None, :]]))
    fc1W, fc1b = _np(params["fc1"][0]), _np(params["fc1"][1])
    put(gpr, g_cols["fc1Wb"], np.vstack([fc1W, fc1b[None, :]]))
    fc2W, fc2b = _np(params["fc2"][0]), _np(params["fc2"][1])
    put(gpr, g_cols["fc2Wb"], np.vstack([fc2W, fc2b[None, :]]))
    put(gpr, g_cols["ones_row"], np.ones((1, 512), np.float32))

    lprs, lpfs = [], []
    inv_n = 1.0 / S
    for i in range(4):
        phi = [( _np(w), _np(b)) for (w, b) in params["phi"][i]]
        psi = [( _np(w), _np(b)) for (w, b) in params["psi"][i]]
        wW, wb = _np(params["w"][i][0]), _np(params["w"][i][1])

        pr = np.zeros((128, R_TOT), np.float32)
        pf = np.zeros((128, F_TOT), np.float32)

        W0ps = np.zeros((2, 128), np.float32)
        W0ps[:, 0:64] = phi[0][0]
        W0ps[:, 64:128] = psi[0][0]
        put(pr, r_cols["W0ps"], W0ps)
        W1blk = np.zeros((128, 256), np.float32)
        W1blk[0:64, 0:128] = phi[1][0]
        W1blk[64:128, 128:256] = psi[1][0]
        put(pr, r_cols["W1blk"], W1blk)
        put(pr, r_cols["W2p"], phi[2][0])
        # fold phi W3: sum over the 128-wide (i,r) block per o
        W3p = phi[3][0].astype(np.float64)
        W3pf = W3p.reshape(256, WIDTH, WIDTH * RANK).sum(-1).astype(np.float32)
        put(pr, r_cols["W3pf"], np.hstack([W3pf[0:128], W3pf[128:256]]))
        put(pr, r_cols["s2W"], psi[2][0])
        put(pr, r_cols["wWb"], np.vstack([wW, wb[None, :]]))

        b0ps = np.concatenate([phi[0][1], psi[0][1]])
        put(pf, f_cols["b0ps"], b0ps[:, None])
        put(pf, f_cols["b1p"], phi[1][1][:, None])
        put(pf, f_cols["b1s"], psi[1][1][:, None])
        put(pf, f_cols["p2b"], phi[2][1].reshape(2, 128).T)
        b3pf = phi[3][1].astype(np.float64).reshape(WIDTH, WIDTH * RANK).sum(-1)
        put(pf, f_cols["b3pf"], b3pf.astype(np.float32)[:, None])
        put(pr, r_cols["b3pf_row"], b3pf.astype(np.float32)[None, :])
        put(pf, f_cols["s2b_rep"],
            np.broadcast_to(psi[2][1], (128, 256)).copy())
        W3s = psi[3][0].astype(np.float64)
        W3sf = (W3s.reshape(256, WIDTH * WIDTH, RANK).sum(-1) * inv_n).astype(np.float32)
        put(pf, f_cols["W3sf"], np.hstack([W3sf[0:128], W3sf[128:256]]))
        b3s = psi[3][1].astype(np.float64)
        b3s_oi = (b3s.reshape(WIDTH * WIDTH, RANK).sum(-1) * inv_n).reshape(WIDTH, WIDTH)
        put(pf, f_cols["b3s_io"], b3s_oi.T.astype(np.float32))

        lprs.append(pr)
        lpfs.append(pf)
    return gpr, gpf, lprs, lpfs


def _run(v, params, trace=False, tmpdir=None):
    from concourse.bass_utils import run_bass_kernel_spmd

    nc = _build()
    v = _np(v)
    gpr, gpf, lprs, lpfs = _pack_weights(params)

    grid = np.linspace(0.0, 1.0, S).astype(np.float32)
    in_maps = []
    for c in range(N_CORES):
        aT = np.empty((3, S), np.float32)
        aT[0] = v[c, NUMI:, 0]
        aT[1] = grid
        aT[2] = 1.0
        m = {"aT": aT, "gpr": gpr, "gpf": gpf}
        for i in range(4):
            m[f"lpr{i}"] = lprs[i]
            m[f"lpf{i}"] = lpfs[i]
        in_maps.append(m)

    res = run_bass_kernel_spmd(nc, in_maps, list(range(N_CORES)), trace=trace,
                               tmpdir=tmpdir)
    out = np.stack([res.results[c]["out"].reshape(S, 1) for c in range(N_CORES)])
    return out, res


def kernel(v, params):
    out, _ = _run(v, params, trace=False)
    return out
